# revision 21
# baseline (speedup 1.0000x reference)
"""EntropyPoolLayer Trainium2 kernel.

Math: out[n,oh,ow,c] = x[n, 2oh+di, 2ow+dj, c] for the window element whose
global value-count is minimal (entropy -p log p is strictly increasing in p
for p < 1/e, and max p ~ 0.04 here, so argmin entropy == argmin count, with
identical tie structure; ties resolved to the first window index k = 2di+dj).

Plan (8 NeuronCores, batch-sharded 4/core, SBUF layout partition = h):
  Phase 1: per-core exact histogram of key = round(10x)+64 in [0,128).
  Host:    merge counts, replicate the reference's f32 entropy per bin,
           rank bins by entropy (equal ent -> equal rank), build packed
           table T'[b] = 65536 - (rank[b]*512 + b).
  Phase 2: per-element acc = T'[key] via a custom 2-bin/pass DVE sweep;
           comparable = acc - 128*k ; max over the 2x2 window (pool_max over
           dj within partitions, stream_shuffle+max over di across adjacent
           partitions) selects lexicographic-min (rank, k); key recovered
           by mod 128, value = (key-64)/10 (exact f32 divide).
"""

import re
import sys

import numpy as np

sys.path.insert(0, "/opt/trn_rl_repo")

import concourse.bacc as bacc  # noqa: E402
import concourse.mybir as mybir  # noqa: E402
import concourse.tile as tile  # noqa: E402
from concourse.bass_utils import run_bass_kernel_spmd  # noqa: E402
from concourse.dve_ops import OPS, DveOp  # noqa: E402
from concourse.dve_spec import (  # noqa: E402
    AluOp,
    C0,
    C1,
    C2,
    One,
    Spec,
    Src0,
    Src1,
    Zero,
    eq,
    maxx,
)

F32 = mybir.dt.float32
I32 = mybir.dt.int32
BF16 = mybir.dt.bfloat16
ALU = mybir.AluOpType

N_CORES = 8
N, H, W, C = 32, 128, 160, 64
NPC = N // N_CORES  # batches per core
OH, OW = H // 2, W // 2
NB = 128  # key bins
BIN_LO = 8  # phase-2 sweep window [BIN_LO, BIN_LO + NSWEEP)
NSWEEP = 112
FREE = W * C  # 10240 free elements per partition per batch
BIG = 65536.0

# --------------------------------------------------------------------------
# Custom DVE op: acc' = max(acc, (k==b)*v0 + (k==b+1)*v1)
# C2 (imm2, compile-time literal) = base bin b; C0/C1 ([P,1] APs) = packed
# table values for bins b and b+1.
# --------------------------------------------------------------------------


def _lut2_ref(in0, in1, s0, s1, imm2):
    k = in0.astype(np.float32)
    val = (k == np.float32(imm2)) * s0 + (k == np.float32(imm2 + 1.0)) * s1
    return np.maximum(in1.astype(np.float32), val)


def _register(name: str, spec: Spec) -> DveOp:
    import concourse.dve_ops as dve_ops

    for op in OPS:
        if op.name == name:
            return op
    tmp = DveOp(name, spec, subdim=False, uops_sha={})
    OPS.append(tmp)
    idx = next(i for i, o in enumerate(OPS) if o.name == name)
    dve_ops._SUB_OPCODE_FOR_NAME[name] = dve_ops._CUSTOM_DVE_ROW_BASE + idx
    dve_ops.CUSTOM_DVE_SPECS[name] = spec
    shas = {}
    for ver in ("v3", "v4"):
        try:
            tmp.compile(ver)
        except ValueError as e:
            m = re.search(r'="([0-9a-f]+)"', str(e))
            if m is None:
                raise
            shas[ver] = m.group(1)
    final = DveOp(name, spec, subdim=False, uops_sha=shas)
    OPS[idx] = final
    return final


def _histo2_ref(in0, in1, s0, s1, imm2):
    k = in0.astype(np.float32)
    body = (k == np.float32(imm2)) + s0 * (k == np.float32(s1))
    return body, body.reshape(body.shape[0], -1).sum(axis=-1, keepdims=True)


LUT2 = _register(
    "ENTROPY_LUT2",
    Spec(
        body=maxx(Src1, eq(Src0, C2) * C0 + eq(Src0, C2 + One) * C1),
        reference=_lut2_ref,
    ),
)

# accum = count(b) + s0 * count(b+1); s0 = 4096, cols/pass <= 4095
HISTO2 = _register(
    "ENTROPY_HISTO2",
    Spec(
        body=eq(Src0, C2) + C0 * eq(Src0, C1),
        accum=AluOp.ADD,
        accum_init=Zero,
        reference=_histo2_ref,
    ),
)

# --------------------------------------------------------------------------
# Kernel builders
# --------------------------------------------------------------------------


def build_phase1():
    """Exact 128-bin histogram via PE joint counting.

    Interleaved indicator planes per element (bf16, {0,1}):
      hi[e*8+s]  = [1, key>=16, ..., key>=112][s]   (thermometer, s=0 is ones)
      lo[e*16+r] = (int(key) & 15) == r
    PE contracts pages of 128 elements, 8 pages per matmul:
      psum[(pg, r), (pg, s)] += lo . hi   accumulated over all pages.
    Host keeps pg-diagonal blocks and differences the thermometer:
      count[16*h + r] = C[h, r] - C[h+1, r].
    """
    nc = bacc.Bacc("TRN2", target_bir_lowering=False, debug=False)
    x = nc.dram_tensor("x", [NPC, H, W, C], F32, kind="ExternalInput")
    counts = nc.dram_tensor("counts", [128, 64], F32, kind="ExternalOutput")
    keys = nc.dram_tensor("keys", [128, NPC * FREE], BF16, kind="ExternalOutput")
    xv = x[:].rearrange("n h w c -> h n (w c)")  # [128, NPC, FREE]
    QC = FREE // 4  # 2560 cols per chunk
    PG = 8  # pages per matmul: lhsT = lo [128, 8*16], rhs = hi [128, 8*8]
    NBLK = QC // PG

    with tile.TileContext(nc) as tc:
        with (
            tc.tile_pool(name="xp", bufs=1) as xp,
            tc.tile_pool(name="kp", bufs=2) as kp,
            tc.tile_pool(name="ip", bufs=1) as ip,
            tc.tile_pool(name="hp", bufs=2) as hp,
            tc.tile_pool(name="lp", bufs=1) as lp,
            tc.tile_pool(name="ps", bufs=1, space="PSUM") as ps,
            tc.tile_pool(name="cp", bufs=1) as cp,
        ):
            psum = ps.tile([128, 64], F32)
            first = True
            for n in range(NPC):
                for q in range(4):
                    xt = xp.tile([128, QC], F32)
                    nc.sync.dma_start(xt[:], xv[:, n, q * QC : (q + 1) * QC])
                    kt = kp.tile([128, QC], BF16)
                    nc.vector.tensor_scalar(
                        kt[:], xt[:], 10.0, 64.0, ALU.mult, ALU.add
                    )
                    off = (n * 4 + q) * QC
                    nc.sync.dma_start(keys[:, off : off + QC], kt[:])
                    ki = ip.tile([128, QC], I32)
                    nc.vector.tensor_copy(ki[:], kt[:])
                    nc.vector.tensor_scalar(ki[:], ki[:], 15, None, ALU.bitwise_and)
                    lt = ip.tile([128, QC], BF16, tag="lt")
                    nc.vector.tensor_copy(lt[:], ki[:])

                    # interleaved planes: hi [128, QC*8], lo [128, QC*16]
                    hi = hp.tile([128, QC * 8], BF16)
                    hiv = hi[:].rearrange("p (c s) -> p s c", s=8)
                    nc.vector.memset(hiv[:, 0, :], 1.0)
                    for s in range(1, 8):
                        nc.vector.tensor_scalar(
                            hiv[:, s, :], kt[:], float(16 * s), None, ALU.is_ge
                        )
                    lo = lp.tile([128, QC * 16], BF16)
                    lov = lo[:].rearrange("p (c r) -> p r c", r=16)
                    for r in range(16):
                        nc.vector.tensor_scalar(
                            lov[:, r, :], lt[:], float(r), None, ALU.is_equal
                        )

                    hb = hi[:].rearrange("p (b gs) -> p b gs", gs=PG * 8)
                    lb = lo[:].rearrange("p (b gr) -> p b gr", gr=PG * 16)
                    for b in range(NBLK):
                        nc.tensor.matmul(
                            psum[:],
                            lb[:, b],
                            hb[:, b],
                            start=first,
                            stop=(n == NPC - 1 and q == 3 and b == NBLK - 1),
                        )
                        first = False
            csb = cp.tile([128, 64], F32)
            nc.vector.tensor_copy(csb[:], psum[:])
            nc.sync.dma_start(counts[:], csb[:])
    nc.compile()
    return nc


def build_phase2():
    nc = bacc.Bacc("TRN2", target_bir_lowering=False, debug=False)
    keys = nc.dram_tensor("keys", [128, NPC * FREE], BF16, kind="ExternalInput")
    values = nc.dram_tensor("values", [128, NB + 1], F32, kind="ExternalInput")
    out = nc.dram_tensor("out", [NPC, OH, OW, C], F32, kind="ExternalOutput")
    ov = out[:].rearrange("n oh ow c -> oh n (ow c)")  # [64, NPC, OW*C]
    HO = OW * C  # pooled cols per chunk (chunk = one batch image)

    with tile.TileContext(nc) as tc:
        with (
            tc.tile_pool(name="kp", bufs=2) as kp,
            tc.tile_pool(name="ap", bufs=1) as ap_,
            tc.tile_pool(name="pp", bufs=1) as pp,
            tc.tile_pool(name="ep", bufs=1) as ep,
            tc.tile_pool(name="const", bufs=1) as constp,
        ):
            vals = constp.tile([128, NB + 1], F32)
            nc.sync.dma_start(vals[:], values[:])
            rbias = vals[:, NB : NB + 1]  # 256*(p%2), host-provided

            # kpos[p, w, c] = 128 * (w%2)
            kpos = constp.tile([128, FREE], BF16)
            kv = kpos[:].rearrange("p (w c) -> p w c", c=C)
            nc.vector.memset(kv[:, 0::2, :], 0.0)
            nc.vector.memset(kv[:, 1::2, :], 128.0)

            for n in range(NPC):
                kt = kp.tile([128, FREE], BF16)
                nc.sync.dma_start(kt[:], keys[:, n * FREE : (n + 1) * FREE])

                acc = ap_.tile([128, FREE], F32)
                nc.vector.memset(acc[:], 0.0)
                for j in range(NSWEEP // 2):
                    b0 = BIN_LO + 2 * j
                    nc.vector._custom_dve(
                        LUT2,
                        out=acc[:],
                        in0=kt[:],
                        in1=acc[:],
                        s0=vals[:, b0 : b0 + 1],
                        s1=vals[:, b0 + 1 : b0 + 2],
                        imm2=float(b0),
                    )
                # comparable = acc - 128*dj - 256*di
                # (max <=> lexicographic min (rank, k), k = 2*di + dj)
                nc.vector.tensor_tensor(acc[:], acc[:], kpos[:], ALU.subtract)

                # max over dj (within partition): TT-max of w-even/w-odd views
                pooled = pp.tile([128, HO], F32)
                av = acc[:].rearrange("p (ow dj c) -> p ow dj c", ow=OW, dj=2, c=C)
                pj = pooled[:].rearrange("p (ow c) -> p ow c", c=C)
                nc.vector.tensor_tensor(pj, av[:, :, 0, :], av[:, :, 1, :], ALU.max)
                nc.vector.tensor_scalar(pooled[:], pooled[:], rbias, None, ALU.subtract)

                # max over di (adjacent partition pairs)
                shuf = pp.tile([128, HO], F32, tag="shuf")
                nc.vector.stream_shuffle(shuf[:], pooled[:], [i ^ 1 for i in range(32)])
                nc.vector.tensor_tensor(pooled[:], pooled[:], shuf[:], ALU.max)

                # u = BIG - m; key = u & 127; v = (key-64)*0.1
                ext = ep.tile([128, HO], F32)
                nc.vector.tensor_scalar(ext[:], pooled[:], -1.0, BIG, ALU.mult, ALU.add)
                exi = ep.tile([128, HO], I32, tag="exi")
                nc.vector.tensor_copy(exi[:], ext[:])
                nc.vector.tensor_scalar(exi[:], exi[:], 127, None, ALU.bitwise_and)
                nc.vector.tensor_copy(ext[:], exi[:])
                nc.vector.tensor_scalar(ext[:], ext[:], -64.0, 0.1, ALU.add, ALU.mult)

                nc.sync.dma_start(ov[:, n, :], ext[0::2, :])
    nc.compile()
    return nc


_CACHE = {}


def _get(name, builder):
    if name not in _CACHE:
        _CACHE[name] = builder()
    return _CACHE[name]


# --------------------------------------------------------------------------
# Host orchestration
# --------------------------------------------------------------------------


def _rank_table(counts: np.ndarray) -> np.ndarray:
    """Replicate the reference's f32 entropy per bin and rank bins by it
    (equal f32 entropy -> equal rank). counts: int64[NB]."""
    size = np.float32(counts.sum())
    present = counts > 0
    p = counts.astype(np.float32) / size  # f32 division, like jnp
    with np.errstate(divide="ignore", invalid="ignore"):
        ent = (-p * np.log(p.astype(np.float32)).astype(np.float32)).astype(np.float32)
    ent[~present] = np.inf
    # rank by entropy ascending; equal ent values share a rank
    uniq = np.unique(ent[present])  # sorted ascending
    rank = np.zeros(NB, dtype=np.int64)
    rank[present] = np.searchsorted(uniq, ent[present])
    return rank, present


def kernel(inputs: np.ndarray) -> np.ndarray:
    x = np.ascontiguousarray(np.asarray(inputs, dtype=np.float32))
    assert x.shape == (N, H, W, C), x.shape

    core_ids = list(range(N_CORES))
    shards = [x[i * NPC : (i + 1) * NPC] for i in range(N_CORES)]

    # ---- phase 1: exact global histogram --------------------------------
    nc1 = _get("p1", build_phase1)
    in_maps = [{"x": s} for s in shards]
    res1 = run_bass_kernel_spmd(nc1, in_maps, core_ids).results
    keys_list = [r["keys"] for r in res1]
    counts = np.zeros(NB, dtype=np.int64)
    for r in res1:
        a = np.round(r["counts"].astype(np.float64)).astype(np.int64)
        a = a.reshape(8, 16, 8, 8)  # [(pg, r), (pg', s)]
        c2 = np.zeros((9, 16), dtype=np.int64)
        c2[:8] = np.einsum("grgs->sr", a)  # pg == pg' diagonal blocks
        cnt = c2[:8] - c2[1:]  # thermometer difference over s
        counts += cnt.reshape(NB)
    total = int(counts.sum())
    assert total == N * H * W * C, (
        f"histogram lost elements: {total} != {N * H * W * C} "
        "(keys outside [0,128)?)"
    )
    # entropy is strictly increasing in count only below p = 1/e
    assert counts.max() / total < 0.3678, "p_max >= 1/e; rank ordering invalid"
    assert counts[:BIN_LO].sum() == 0 and counts[BIN_LO + NSWEEP :].sum() == 0, (
        "keys outside the phase-2 sweep window"
    )

    # ---- host: packed table ---------------------------------------------
    rank, present = _rank_table(counts)
    assert rank.max() * 512 + NB <= 65535
    tbl = np.zeros(NB, dtype=np.float32)
    b = np.arange(NB)
    tbl[present] = BIG - (rank[present] * 512 + b[present]).astype(np.float32)
    values_np = np.zeros((128, NB + 1), dtype=np.float32)
    values_np[:, :NB] = tbl
    values_np[1::2, NB] = 256.0  # rbias: 256*(h%2) for the di tie-break

    # ---- phase 2: pooling ------------------------------------------------
    nc2 = _get("p2", build_phase2)
    in_maps2 = [
        {"keys": k, "values": values_np} for k in keys_list
    ]
    res2 = run_bass_kernel_spmd(nc2, in_maps2, core_ids).results
    out = np.concatenate([r["out"] for r in res2], axis=0)
    assert out.shape == (N, OH, OW, C)
    return out


# revision 22
# speedup vs baseline: 1.0128x; 1.0128x over previous
"""EntropyPoolLayer Trainium2 kernel.

Math: out[n,oh,ow,c] = x[n, 2oh+di, 2ow+dj, c] for the window element whose
global value-count is minimal (entropy -p log p is strictly increasing in p
for p < 1/e, and max p ~ 0.04 here, so argmin entropy == argmin count, with
identical tie structure; ties resolved to the first window index k = 2di+dj).

Plan (8 NeuronCores, batch-sharded 4/core, SBUF layout partition = h):
  Phase 1: per-core exact histogram of key = round(10x)+64 in [0,128).
  Host:    merge counts, replicate the reference's f32 entropy per bin,
           rank bins by entropy (equal ent -> equal rank), build packed
           table T'[b] = 65536 - (rank[b]*512 + b).
  Phase 2: per-element acc = T'[key] via a custom 2-bin/pass DVE sweep;
           comparable = acc - 128*k ; max over the 2x2 window (pool_max over
           dj within partitions, stream_shuffle+max over di across adjacent
           partitions) selects lexicographic-min (rank, k); key recovered
           by mod 128, value = (key-64)/10 (exact f32 divide).
"""

import re
import sys

import numpy as np

sys.path.insert(0, "/opt/trn_rl_repo")

import concourse.bacc as bacc  # noqa: E402
import concourse.mybir as mybir  # noqa: E402
import concourse.tile as tile  # noqa: E402
from concourse.bass_utils import run_bass_kernel_spmd  # noqa: E402
from concourse.dve_ops import OPS, DveOp  # noqa: E402
from concourse.dve_spec import (  # noqa: E402
    AluOp,
    C0,
    C1,
    C2,
    One,
    Spec,
    Src0,
    Src1,
    Zero,
    eq,
    maxx,
)

F32 = mybir.dt.float32
I32 = mybir.dt.int32
BF16 = mybir.dt.bfloat16
ALU = mybir.AluOpType

N_CORES = 8
N, H, W, C = 32, 128, 160, 64
NPC = N // N_CORES  # batches per core
OH, OW = H // 2, W // 2
NB = 128  # key bins
BIN_LO = 8  # phase-2 sweep window [BIN_LO, BIN_LO + NSWEEP)
NSWEEP = 112
FREE = W * C  # 10240 free elements per partition per batch
BIG = 65536.0

# --------------------------------------------------------------------------
# Custom DVE op: acc' = max(acc, (k==b)*v0 + (k==b+1)*v1)
# C2 (imm2, compile-time literal) = base bin b; C0/C1 ([P,1] APs) = packed
# table values for bins b and b+1.
# --------------------------------------------------------------------------


def _lut2_ref(in0, in1, s0, s1, imm2):
    k = in0.astype(np.float32)
    val = (k == np.float32(imm2)) * s0 + (k == np.float32(imm2 + 1.0)) * s1
    return np.maximum(in1.astype(np.float32), val)


def _register(name: str, spec: Spec) -> DveOp:
    import concourse.dve_ops as dve_ops

    for op in OPS:
        if op.name == name:
            return op
    tmp = DveOp(name, spec, subdim=False, uops_sha={})
    OPS.append(tmp)
    idx = next(i for i, o in enumerate(OPS) if o.name == name)
    dve_ops._SUB_OPCODE_FOR_NAME[name] = dve_ops._CUSTOM_DVE_ROW_BASE + idx
    dve_ops.CUSTOM_DVE_SPECS[name] = spec
    shas = {}
    for ver in ("v3", "v4"):
        try:
            tmp.compile(ver)
        except ValueError as e:
            m = re.search(r'="([0-9a-f]+)"', str(e))
            if m is None:
                raise
            shas[ver] = m.group(1)
    final = DveOp(name, spec, subdim=False, uops_sha=shas)
    OPS[idx] = final
    return final


def _histo2_ref(in0, in1, s0, s1, imm2):
    k = in0.astype(np.float32)
    body = (k == np.float32(imm2)) + s0 * (k == np.float32(s1))
    return body, body.reshape(body.shape[0], -1).sum(axis=-1, keepdims=True)


LUT2 = _register(
    "ENTROPY_LUT2",
    Spec(
        body=maxx(Src1, eq(Src0, C2) * C0 + eq(Src0, C2 + One) * C1),
        reference=_lut2_ref,
    ),
)

# accum = count(b) + s0 * count(b+1); s0 = 4096, cols/pass <= 4095
HISTO2 = _register(
    "ENTROPY_HISTO2",
    Spec(
        body=eq(Src0, C2) + C0 * eq(Src0, C1),
        accum=AluOp.ADD,
        accum_init=Zero,
        reference=_histo2_ref,
    ),
)

# --------------------------------------------------------------------------
# Kernel builders
# --------------------------------------------------------------------------


def build_phase1():
    """Exact 128-bin histogram via PE joint counting.

    Interleaved indicator planes per element (bf16, {0,1}):
      hi[e*8+s]  = [1, key>=16, ..., key>=112][s]   (thermometer, s=0 is ones)
      lo[e*16+r] = (int(key) & 15) == r
    PE contracts pages of 128 elements, 8 pages per matmul:
      psum[(pg, r), (pg, s)] += lo . hi   accumulated over all pages.
    Host keeps pg-diagonal blocks and differences the thermometer:
      count[16*h + r] = C[h, r] - C[h+1, r].
    """
    nc = bacc.Bacc("TRN2", target_bir_lowering=False, debug=False)
    x = nc.dram_tensor("x", [NPC, H, W, C], F32, kind="ExternalInput")
    counts = nc.dram_tensor("counts", [128, 64], F32, kind="ExternalOutput")
    keys = nc.dram_tensor("keys", [128, NPC * FREE], BF16, kind="ExternalOutput")
    xv = x[:].rearrange("n h w c -> h n (w c)")  # [128, NPC, FREE]
    QC = FREE // 4  # 2560 cols per chunk
    PG = 8  # pages per matmul: lhsT = lo [128, 8*16], rhs = hi [128, 8*8]
    NBLK = QC // PG

    with tile.TileContext(nc) as tc:
        with (
            tc.tile_pool(name="xp", bufs=1) as xp,
            tc.tile_pool(name="kp", bufs=2) as kp,
            tc.tile_pool(name="ip", bufs=1) as ip,
            tc.tile_pool(name="hp", bufs=2) as hp,
            tc.tile_pool(name="lp", bufs=1) as lp,
            tc.tile_pool(name="ps", bufs=1, space="PSUM") as ps,
            tc.tile_pool(name="cp", bufs=1) as cp,
        ):
            psum = ps.tile([128, 64], F32)
            first = True
            for n in range(NPC):
                for q in range(4):
                    xt = xp.tile([128, QC], F32)
                    nc.sync.dma_start(xt[:], xv[:, n, q * QC : (q + 1) * QC])
                    kt = kp.tile([128, QC], BF16)
                    nc.vector.tensor_scalar(
                        kt[:], xt[:], 10.0, 64.0, ALU.mult, ALU.add
                    )
                    off = (n * 4 + q) * QC
                    nc.sync.dma_start(keys[:, off : off + QC], kt[:])
                    ki = ip.tile([128, QC], I32)
                    nc.vector.tensor_copy(ki[:], kt[:])
                    nc.vector.tensor_scalar(ki[:], ki[:], 15, None, ALU.bitwise_and)
                    lt = ip.tile([128, QC], BF16, tag="lt")
                    nc.vector.tensor_copy(lt[:], ki[:])

                    # interleaved planes: hi [128, QC*8], lo [128, QC*16]
                    hi = hp.tile([128, QC * 8], BF16)
                    hiv = hi[:].rearrange("p (c s) -> p s c", s=8)
                    nc.vector.memset(hiv[:, 0, :], 1.0)
                    for s in range(1, 8):
                        nc.vector.tensor_scalar(
                            hiv[:, s, :], kt[:], float(16 * s), None, ALU.is_ge
                        )
                    lo = lp.tile([128, QC * 16], BF16)
                    lov = lo[:].rearrange("p (c r) -> p r c", r=16)
                    for r in range(16):
                        nc.vector.tensor_scalar(
                            lov[:, r, :], lt[:], float(r), None, ALU.is_equal
                        )

                    hb = hi[:].rearrange("p (b gs) -> p b gs", gs=PG * 8)
                    lb = lo[:].rearrange("p (b gr) -> p b gr", gr=PG * 16)
                    for b in range(NBLK):
                        nc.tensor.matmul(
                            psum[:],
                            lb[:, b],
                            hb[:, b],
                            start=first,
                            stop=(n == NPC - 1 and q == 3 and b == NBLK - 1),
                        )
                        first = False
            csb = cp.tile([128, 64], F32)
            nc.vector.tensor_copy(csb[:], psum[:])
            nc.sync.dma_start(counts[:], csb[:])
    nc.compile()
    return nc


def build_phase2():
    nc = bacc.Bacc("TRN2", target_bir_lowering=False, debug=False)
    keys = nc.dram_tensor("keys", [128, NPC * FREE], BF16, kind="ExternalInput")
    values = nc.dram_tensor("values", [128, NB + 1], F32, kind="ExternalInput")
    out = nc.dram_tensor("out", [NPC, OH, OW, C], F32, kind="ExternalOutput")
    ov = out[:].rearrange("n oh ow c -> oh n (ow c)")  # [64, NPC, OW*C]
    HO = OW * C  # pooled cols per chunk (chunk = one batch image)

    with tile.TileContext(nc) as tc:
        with (
            tc.tile_pool(name="kp", bufs=2) as kp,
            tc.tile_pool(name="ap", bufs=1) as ap_,
            tc.tile_pool(name="pp", bufs=1) as pp,
            tc.tile_pool(name="ep", bufs=1) as ep,
            tc.tile_pool(name="const", bufs=1) as constp,
        ):
            vals = constp.tile([128, NB + 1], F32)
            nc.sync.dma_start(vals[:], values[:])
            rbias = vals[:, NB : NB + 1]  # 256*(p%2), host-provided

            # kpos[p, w, c] = 128 * (w%2)
            kpos = constp.tile([128, FREE], BF16)
            kv = kpos[:].rearrange("p (w c) -> p w c", c=C)
            nc.vector.memset(kv[:, 0::2, :], 0.0)
            nc.vector.memset(kv[:, 1::2, :], 128.0)

            for n in range(NPC):
                kt = kp.tile([128, FREE], BF16)
                nc.sync.dma_start(kt[:], keys[:, n * FREE : (n + 1) * FREE])

                acc = ap_.tile([128, FREE], F32)
                nc.gpsimd.memset(acc[:], 0.0)
                for j in range(NSWEEP // 2):
                    b0 = BIN_LO + 2 * j
                    nc.vector._custom_dve(
                        LUT2,
                        out=acc[:],
                        in0=kt[:],
                        in1=acc[:],
                        s0=vals[:, b0 : b0 + 1],
                        s1=vals[:, b0 + 1 : b0 + 2],
                        imm2=float(b0),
                    )
                # comparable = acc - 128*dj - 256*di
                # (max <=> lexicographic min (rank, k), k = 2*di + dj)
                nc.vector.tensor_tensor(acc[:], acc[:], kpos[:], ALU.subtract)

                # max over dj (within partition): TT-max of w-even/w-odd views
                pooled = pp.tile([128, HO], F32)
                av = acc[:].rearrange("p (ow dj c) -> p ow dj c", ow=OW, dj=2, c=C)
                pj = pooled[:].rearrange("p (ow c) -> p ow c", c=C)
                nc.vector.tensor_tensor(pj, av[:, :, 0, :], av[:, :, 1, :], ALU.max)
                nc.vector.tensor_scalar(pooled[:], pooled[:], rbias, None, ALU.subtract)

                # max over di (adjacent partition pairs)
                shuf = pp.tile([128, HO], F32, tag="shuf")
                nc.vector.stream_shuffle(shuf[:], pooled[:], [i ^ 1 for i in range(32)])
                nc.vector.tensor_tensor(pooled[:], pooled[:], shuf[:], ALU.max)

                # u = BIG - m; key = u & 127; v = (key-64)*0.1
                ext = ep.tile([128, HO], F32)
                nc.vector.tensor_scalar(ext[:], pooled[:], -1.0, BIG, ALU.mult, ALU.add)
                exi = ep.tile([128, HO], I32, tag="exi")
                nc.vector.tensor_copy(exi[:], ext[:])
                nc.vector.tensor_scalar(exi[:], exi[:], 127, None, ALU.bitwise_and)
                nc.vector.tensor_copy(ext[:], exi[:])
                nc.vector.tensor_scalar(ext[:], ext[:], -64.0, 0.1, ALU.add, ALU.mult)

                nc.sync.dma_start(ov[:, n, :], ext[0::2, :])
    nc.compile()
    return nc


_CACHE = {}


def _get(name, builder):
    if name not in _CACHE:
        _CACHE[name] = builder()
    return _CACHE[name]


# --------------------------------------------------------------------------
# Host orchestration
# --------------------------------------------------------------------------


def _rank_table(counts: np.ndarray) -> np.ndarray:
    """Replicate the reference's f32 entropy per bin and rank bins by it
    (equal f32 entropy -> equal rank). counts: int64[NB]."""
    size = np.float32(counts.sum())
    present = counts > 0
    p = counts.astype(np.float32) / size  # f32 division, like jnp
    with np.errstate(divide="ignore", invalid="ignore"):
        ent = (-p * np.log(p.astype(np.float32)).astype(np.float32)).astype(np.float32)
    ent[~present] = np.inf
    # rank by entropy ascending; equal ent values share a rank
    uniq = np.unique(ent[present])  # sorted ascending
    rank = np.zeros(NB, dtype=np.int64)
    rank[present] = np.searchsorted(uniq, ent[present])
    return rank, present


def kernel(inputs: np.ndarray) -> np.ndarray:
    x = np.ascontiguousarray(np.asarray(inputs, dtype=np.float32))
    assert x.shape == (N, H, W, C), x.shape

    core_ids = list(range(N_CORES))
    shards = [x[i * NPC : (i + 1) * NPC] for i in range(N_CORES)]

    # ---- phase 1: exact global histogram --------------------------------
    nc1 = _get("p1", build_phase1)
    in_maps = [{"x": s} for s in shards]
    res1 = run_bass_kernel_spmd(nc1, in_maps, core_ids).results
    keys_list = [r["keys"] for r in res1]
    counts = np.zeros(NB, dtype=np.int64)
    for r in res1:
        a = np.round(r["counts"].astype(np.float64)).astype(np.int64)
        a = a.reshape(8, 16, 8, 8)  # [(pg, r), (pg', s)]
        c2 = np.zeros((9, 16), dtype=np.int64)
        c2[:8] = np.einsum("grgs->sr", a)  # pg == pg' diagonal blocks
        cnt = c2[:8] - c2[1:]  # thermometer difference over s
        counts += cnt.reshape(NB)
    total = int(counts.sum())
    assert total == N * H * W * C, (
        f"histogram lost elements: {total} != {N * H * W * C} "
        "(keys outside [0,128)?)"
    )
    # entropy is strictly increasing in count only below p = 1/e
    assert counts.max() / total < 0.3678, "p_max >= 1/e; rank ordering invalid"
    assert counts[:BIN_LO].sum() == 0 and counts[BIN_LO + NSWEEP :].sum() == 0, (
        "keys outside the phase-2 sweep window"
    )

    # ---- host: packed table ---------------------------------------------
    rank, present = _rank_table(counts)
    assert rank.max() * 512 + NB <= 65535
    tbl = np.zeros(NB, dtype=np.float32)
    b = np.arange(NB)
    tbl[present] = BIG - (rank[present] * 512 + b[present]).astype(np.float32)
    values_np = np.zeros((128, NB + 1), dtype=np.float32)
    values_np[:, :NB] = tbl
    values_np[1::2, NB] = 256.0  # rbias: 256*(h%2) for the di tie-break

    # ---- phase 2: pooling ------------------------------------------------
    nc2 = _get("p2", build_phase2)
    in_maps2 = [
        {"keys": k, "values": values_np} for k in keys_list
    ]
    res2 = run_bass_kernel_spmd(nc2, in_maps2, core_ids).results
    out = np.concatenate([r["out"] for r in res2], axis=0)
    assert out.shape == (N, OH, OW, C)
    return out
